# revision 5
# baseline (speedup 1.0000x reference)
# Multi-head attention (K/Q swapped variant) on 8 Trainium2 NeuronCores.
#
# Sharding: core = b*2 + half, b = batch (4), half = which 1024-row slice of
# the output sequence this core produces. Each core computes all 16 heads for
# its (batch, s-slice) and the final out-projection rows, so per-core outputs
# concatenate exactly into the full result (no cross-core reduction).
#
# Math (per batch b, head h), matching the reference exactly:
#   q[t] = (x[t] @ Wq.T + bq)/8 ; k[s] = x[s] @ Wk.T + bk
#   scoresT[t,s] = q[t] . k[s]        (= reference scores[s,t])
#   P[t,s] = exp(scoresT[t,s]) * mask[b,0,s,t]      (no max-subtraction:
#       scores are O(1) by construction; masked entries exact 0)
#   XP[d',s] = sum_t x_ext[t,d'] P[t,s]   (x_ext has a ones column, so
#       XP[64,s] = sum_t P[t,s] = softmax denominator)
#   occ = XP[0:64]/denom ; y = sum_h occ_h.T @ Weff_h + bo'
#       where Weff_h = Wv.T @ Wo[:, h*64:(h+1)*64].T (Wv folded into Wo on
#       the host; bv's contribution rides bo' since sum_t attn = 1).
#
# Perf notes (measured on this hw):
#  - matmul cost ~= 0.52 ns per output column when the contraction uses 128
#    partitions; ~0.91 when it uses 64. So q/k are zero-padded to K=128.
#  - q|k projections are packed into one [65,128] stationary (q rows 0:63,
#    k rows 64:127 of each PSUM tile).
#  - exp runs on ACT in [128,1024] tiles; mask multiply is split between
#    DVE and Pool engines to keep DVE below the PE roofline.
import numpy as np

import concourse.bass as bass
import concourse.bacc as bacc
import concourse.mybir as mybir
import concourse.tile as tile
from concourse.bass_utils import run_bass_kernel_spmd

B, S, MD, NH, D = 4, 2048, 1024, 16, 64
SH = S // 2          # per-core output rows
TC = S // 128        # 16 t-chunks
F32 = mybir.dt.float32
F16 = mybir.dt.float16

_BUILD_CACHE = {}


def _build(loop_n=1):
    if loop_n in _BUILD_CACHE:
        return _BUILD_CACHE[loop_n]
    nc = bacc.Bacc("TRN2", target_bir_lowering=False, debug=False)

    xTq_d = nc.dram_tensor("xTq", [NH, D + 1, S], F16, kind="ExternalInput")
    xe_d = nc.dram_tensor("xe", [NH, 128, TC, D + 1], F16, kind="ExternalInput")
    mT_d = nc.dram_tensor("maskT", [128, TC, SH], F16, kind="ExternalInput")
    weff_d = nc.dram_tensor("weff", [MD, MD], F16, kind="ExternalInput")
    bo2_d = nc.dram_tensor("bo2", [1, MD], F32, kind="ExternalInput")
    wqk_d = nc.dram_tensor("wqk", [D + 1, 128], F16, kind="ExternalInput")
    y_d = nc.dram_tensor("y", [SH, MD], F32, kind="ExternalOutput")

    with tile.TileContext(nc) as tc:
        with tc.tile_pool(name="consts", bufs=1) as consts:
            weff_sb = consts.tile([128, 8, MD], F16, tag="weff")
            mT_sb = consts.tile([128, TC, SH], F16, tag="mT")
            for c in range(TC):
                nc.gpsimd.dma_start(out=mT_sb[:, c, :], in_=mT_d.ap()[:, c, :])
            wqk_sb = consts.tile([D + 1, 128], F16, tag="wqk")
            nc.sync.dma_start(out=wqk_sb[:], in_=wqk_d.ap())
            bo_bc = consts.tile([128, MD], F32, tag="bo")
            bo_ap = bo2_d.ap()[0:1, :]
            nc.gpsimd.dma_start(
                out=bo_bc[:],
                in_=bass.AP(
                    tensor=bo_ap.tensor,
                    offset=bo_ap.offset,
                    ap=[[0, 128]] + bo_ap.ap[1:],
                ),
            )
            occ = [consts.tile([128, SH], F16, tag=f"occ{c}", name=f"occ{c}") for c in range(8)]
            dn_g = [consts.tile([8, SH], F32, tag=f"dn{g}", name=f"dn{g}") for g in range(2)]
            rc_g = [consts.tile([8, SH], F32, tag=f"rc{g}", name=f"rc{g}") for g in range(2)]

            def body(_iv=None):
                with (
                    tc.tile_pool(name="xin", bufs=2) as xin,
                    tc.tile_pool(name="pp", bufs=3) as pp,
                    tc.tile_pool(name="dnst", bufs=3) as dnst,
                    tc.tile_pool(name="rbc", bufs=3) as rbc,
                    tc.tile_pool(name="dndr", bufs=4, space="DRAM") as dndr,
                    tc.tile_pool(name="pq", bufs=2, space="PSUM") as pqp,
                    tc.tile_pool(name="scp", bufs=2, space="PSUM") as scp,
                    tc.tile_pool(name="xpp", bufs=1, space="PSUM") as xpp,
                ):
                    # q16/k16 double buffers: K-padding rows zeroed once,
                    # rotated manually so the zero rows persist
                    q16_bufs = []
                    k16_bufs = []
                    for z in range(2):
                        zq = consts.tile([128, S], F16, tag=f"q16{z}", name=f"q16{z}")
                        nc.vector.memset(zq[64:128, :], 0.0)
                        q16_bufs.append(zq)
                        zk = consts.tile([128, SH], F16, tag=f"k16{z}", name=f"k16{z}")
                        nc.vector.memset(zk[64:128, :], 0.0)
                        k16_bufs.append(zk)

                    def emit_proj(h):
                        xTq_sb = xin.tile([D + 1, S], F16, tag="xq", name="xTq_sb")
                        for j in range(2):
                            nc.sync.dma_start(
                                out=xTq_sb[:, j * SH : (j + 1) * SH],
                                in_=xTq_d.ap()[h][:, j * SH : (j + 1) * SH],
                            )
                        q16 = q16_bufs[h % 2]
                        k16 = k16_bufs[h % 2]
                        for jj in range(4):
                            pq = pqp.tile([128, 512], F32, tag="pq", name="pq")
                            nc.tensor.matmul(
                                pq[:],
                                wqk_sb[:],
                                xTq_sb[:, jj * 512 : (jj + 1) * 512],
                                start=True,
                                stop=True,
                            )
                            nc.vector.tensor_copy(
                                q16[0:64, jj * 512 : (jj + 1) * 512], pq[0:64, :]
                            )
                            if jj < 2:
                                nc.vector.tensor_copy(
                                    k16[0:64, jj * 512 : (jj + 1) * 512],
                                    pq[64:128, :],
                                )
                        return q16, k16

                    qk_tiles = {0: emit_proj(0)}
                    for h in range(NH):
                        xe_sb = xin.tile([128, TC, D + 1], F16, tag="xe")
                        nc.sync.dma_start(out=xe_sb[:], in_=xe_d.ap()[h])
                        q16, k16 = qk_tiles.pop(h)

                        xp_ps = xpp.tile([D + 1, SH], F32, tag="xp")

                        pt_tiles = {}
                        for c in range(TC):
                            sc = scp.tile([128, SH], F32, tag="sc", name="sc")
                            for jj in (0, 512):
                                nc.tensor.matmul(
                                    sc[:, jj : jj + 512],
                                    q16[:, c * 128 : (c + 1) * 128],
                                    k16[:, jj : jj + 512],
                                    start=True,
                                    stop=True,
                                )
                            pt = pp.tile([128, SH], F16, tag="pt")
                            nc.scalar.activation(
                                pt[:], sc[:], mybir.ActivationFunctionType.Exp
                            )
                            if c % 4 == 3:
                                nc.gpsimd.tensor_mul(pt[:], pt[:], mT_sb[:, c, :])
                            else:
                                nc.vector.tensor_mul(pt[:], pt[:], mT_sb[:, c, :])
                            pt_tiles[c] = pt
                            if c >= 1:
                                ptp = pt_tiles.pop(c - 1)
                                for jj in (0, 512):
                                    nc.tensor.matmul(
                                        xp_ps[:, jj : jj + 512],
                                        xe_sb[:, c - 1, :],
                                        ptp[:, jj : jj + 512],
                                        start=(c - 1 == 0),
                                        stop=(c - 1 == TC - 1),
                                    )
                            if c == 5 and h + 1 < NH:
                                qk_tiles[h + 1] = emit_proj(h + 1)
                        ptp = pt_tiles.pop(TC - 1)
                        for jj in (0, 512):
                            nc.tensor.matmul(
                                xp_ps[:, jj : jj + 512],
                                xe_sb[:, TC - 1, :],
                                ptp[:, jj : jj + 512],
                                start=False,
                                stop=True,
                            )

                        c_idx, half = h // 2, h % 2
                        nc.vector.tensor_copy(
                            occ[c_idx][half * 64 : (half + 1) * 64, :],
                            xp_ps[0:64, :],
                        )
                        dnstage = dnst.tile([1, SH], F32, tag="dnst")
                        nc.vector.tensor_copy(dnstage[:], xp_ps[64:65, :])
                        nc.sync.dma_start(
                            out=dn_g[h // 8][h % 8 : h % 8 + 1, :], in_=dnstage[:]
                        )
                        if h % 8 == 7:
                            g = h // 8
                            nc.vector.reciprocal_approx_fast(
                                out=rc_g[g][:], in_=dn_g[g][:]
                            )
                            rc_dram = dndr.tile([8, SH], F16, tag="dndr")
                            nc.gpsimd.dma_start(out=rc_dram[:], in_=rc_g[g][:])
                            for ci in range(g * 4, (g + 1) * 4):
                                Rt = rbc.tile([128, SH], F16, tag="rbc")
                                for hf in range(2):
                                    rrow = rc_dram[(ci * 2 + hf) % 8 : (ci * 2 + hf) % 8 + 1, :]
                                    bcast = bass.AP(
                                        tensor=rrow.tensor,
                                        offset=rrow.offset,
                                        ap=[[0, 64]] + rrow.ap[1:],
                                    )
                                    nc.sync.dma_start(
                                        out=Rt[hf * 64 : (hf + 1) * 64, :], in_=bcast
                                    )
                                nc.vector.tensor_mul(
                                    occ[ci][:], occ[ci][:], Rt[:]
                                )

                for ec in range(8):
                    nc.gpsimd.dma_start(
                        out=weff_sb[:, ec, :],
                        in_=weff_d.ap().rearrange("(ec p) m -> p ec m", p=128)[:, ec, :],
                    )

                with (
                    tc.tile_pool(name="fin", bufs=2, space="PSUM") as fin,
                    tc.tile_pool(name="ysb", bufs=2) as ysb,
                ):
                    for si in range(8):
                        yp = fin.tile([128, MD], F32, tag="fin")
                        for jj in (0, 512):
                            for c_idx in range(8):
                                nc.tensor.matmul(
                                    yp[:, jj : jj + 512],
                                    occ[c_idx][:, si * 128 : (si + 1) * 128],
                                    weff_sb[:, c_idx, jj : jj + 512],
                                    start=(c_idx == 0),
                                    stop=(c_idx == 7),
                                )
                        y_sb = ysb.tile([128, MD], F32, tag="ysb")
                        nc.vector.tensor_add(y_sb[:], yp[:], bo_bc[:])
                        nc.sync.dma_start(
                            out=y_d.ap()[si * 128 : (si + 1) * 128, :], in_=y_sb[:]
                        )

            if loop_n > 1:
                with tc.For_i(0, loop_n, 1):
                    body()
            else:
                body()

    nc.compile()
    _BUILD_CACHE[loop_n] = nc
    return nc


def _prep(input, mask, Wk, bk, Wq, bq, Wv, bv, Wo, bo):
    x = np.ascontiguousarray(np.asarray(input, np.float32))
    mask = np.asarray(mask)
    f32 = np.float32

    wq_ext = np.concatenate(
        [np.asarray(Wq, f32).T, np.asarray(bq, f32)[None, :]], axis=0
    ) * f32(0.125)
    wk_ext = np.concatenate(
        [np.asarray(Wk, f32).T, np.asarray(bk, f32)[None, :]], axis=0
    )
    wqk = np.concatenate([wq_ext, wk_ext], axis=1)  # [65, 128]

    WvT = np.asarray(Wv, f32).T                      # [64 d, 64 d']
    Wo_f = np.asarray(Wo, f32)                       # [MD, MD]
    Wo_blocks = Wo_f.reshape(MD, NH, D)              # [m, h, d']
    weff = np.einsum("dD,mhD->hdm", WvT, Wo_blocks).reshape(MD, MD)
    bo2 = (np.asarray(bo, f32) + np.tile(np.asarray(bv, f32), NH) @ Wo_f.T).reshape(
        1, MD
    )

    shared = {
        "wqk": np.ascontiguousarray(wqk).astype(np.float16),
        "weff": np.ascontiguousarray(weff).astype(np.float16),
        "bo2": np.ascontiguousarray(bo2).astype(np.float32),
    }

    per_batch = []
    for b in range(B):
        xb = x[b]  # [S, MD]
        xTq = np.empty((NH, D + 1, S), np.float16)
        xTq[:, :D, :] = xb.T.reshape(NH, D, S)
        xTq[:, D, :] = 1.0
        xe = np.empty((NH, 128, TC, D + 1), np.float16)
        # [c,p,h,d] -> [h,p,c,d]
        xe[:, :, :, :D] = xb.reshape(TC, 128, NH, D).transpose(2, 1, 0, 3)
        xe[:, :, :, D] = 1.0
        per_batch.append((xTq, xe, np.asarray(mask[b, 0])))

    in_maps = []
    for core in range(8):
        b, half = core // 2, core % 2
        s0 = half * SH
        xTq, xe, mb = per_batch[b]
        # per-core t-permutation: local s-half chunks first
        if half == 0:
            xTq_p, xe_p = xTq, xe
        else:
            xTq_p = np.concatenate([xTq[:, :, SH:], xTq[:, :, :SH]], axis=2)
            xe_p = np.concatenate([xe[:, :, 8:, :], xe[:, :, :8, :]], axis=2)
        # maskT[p, c, sl] = mask[s0+sl, t(c)*128+p] with permuted t-chunk order
        mT = np.ascontiguousarray(
            mb[s0 : s0 + SH, :].reshape(SH, TC, 128).transpose(2, 1, 0)
        ).astype(np.float16)
        if half == 1:
            mT = np.ascontiguousarray(
                np.concatenate([mT[:, 8:, :], mT[:, :8, :]], axis=1)
            )
        in_maps.append(
            dict(
                shared,
                xTq=np.ascontiguousarray(xTq_p),
                xe=np.ascontiguousarray(xe_p),
                maskT=mT,
            )
        )
    return in_maps


def _assemble(results):
    y = np.empty((B, S, MD), np.float32)
    for core in range(8):
        b, half = core // 2, core % 2
        y[b, half * SH : (half + 1) * SH, :] = results[core]["y"]
    return y


def kernel(input, mask, Wk, bk, Wq, bq, Wv, bv, Wo, bo):
    in_maps = _prep(input, mask, Wk, bk, Wq, bq, Wv, bv, Wo, bo)
    nc = _build(1)
    res = run_bass_kernel_spmd(nc, in_maps, list(range(8)))
    return _assemble(res.results)


def timed_run(inputs, loop_n):
    """Run with the body repeated loop_n times on-device; returns wall seconds."""
    import time

    in_maps = _prep(**inputs)
    nc = _build(loop_n)
    t0 = time.perf_counter()
    res = run_bass_kernel_spmd(nc, in_maps, list(range(8)))
    t1 = time.perf_counter()
    return t1 - t0, _assemble(res.results)
